# revision 9
# baseline (speedup 1.0000x reference)
"""ASTRF kernel for Trainium2 (8 NeuronCores, axon) — fp8 DoubleRow version.

Math: out[b,o,t] = sum_{i,w} weight[o,i,w] * xs[b,i,t-w] + bias[o]
where xs[b,i,src[b,s]] = x[b,i,s] (scatter of events to onsets).

Same banded block-matmul decomposition as the fp32r baseline (t = 64m + q,
q = 8c + q', 9 K=128 chunk-products per 512-wide output half), but each
operand is split hi/lo into a pair of fp8(e4m3) tensors and the three
significant sub-products (Xh*Wh, Xh*Wl, Xl*Wh) are computed with
MatmulPerfMode.DoubleRow (two K=128 slots per instruction at 0.5
cycles/row).  Per half: 4 main-pair DRs + 1 leftover DR (Xl*Wl + Xh*Wh)
+ 9 correction DRs (Xh*Wl + Xl*Wh) = 14 DR matmuls.  Outputs drain as
bf16; host applies the 2^-16 split prescale and the bias.

SBUF layouts (chunk dim stored reversed, a_hat = 7-a, so the banded
product list ascends in memory; plane orders make every DR slot pair a
positive-stride AP):
  x_sb  [128 p=(u_lo,i), j, a_hat, v3=(l,h,l), col]  fp8
  wsh_sb[128 p=(u',i),  win9, v2=(l,h), 512]         fp8
"""

import sys

for _p in ("/opt/trn_rl_repo", "/root/.axon_site/_ro/trn_rl_repo"):
    if _p not in sys.path:
        sys.path.insert(0, _p)

import numpy as np

B, I, S = 4, 16, 4096
O, W = 64, 64
T = 32768
NBLK = T // 64            # 512 blocks per batch
N_CORES = 8
XS, WS = 16.0, 4096.0     # pow2 prescales for fp8 quantization
XCOL = 144                # x col dim padded so DR slot strides are 16-aligned
OUT_SCALE = 1.0 / (XS * WS)
N_WARM = 6
GRP = 4                   # halves interleaved in the DMA-paced phase 1

_prog_cache = {}


def _mats_for(c):
    """Banded product list for half c: (a, k, col0), t-th entry has window
    k = t.  seg1 = t in [0,c] (col0=1, current block), seg2 = t in [c+1,8]
    (col0=0, previous block).  With a_hat = 7-a, a_hat ascends within each
    segment."""
    mats = [(c, 0, 1)]
    mats += [(c - k, k, 1) for k in range(1, c + 1)]
    mats += [(c + 8 - k, k, 0) for k in range(c + 1, 9)]
    return mats


def _half_plan(c):
    """Return (main_pairs, leftover, c1c2) for half c.

    main_pairs: list of (t, ahat, col0) meaning DR over entries (t, t+1)
    leftover:   (t, ahat, col0) main folded into an (Xl*Wl, Xh*Wh) DR
    c1c2:       list of (t, ahat, col0) correction DRs (Xh*Wl, Xl*Wh)
    """
    mats = _mats_for(c)
    ahat = [7 - a for (a, k, col) in mats]
    col = [cc for (a, k, cc) in mats]
    seg1 = list(range(0, c + 1))
    seg2 = list(range(c + 1, 9))
    pairs, leftover = [], None
    for seg in (seg1, seg2):
        for idx in range(0, len(seg) - 1, 2):
            t = seg[idx]
            pairs.append((t, ahat[t], col[t]))
        if len(seg) % 2:
            t = seg[-1]
            leftover = (t, ahat[t], col[t])
    assert len(pairs) == 4 and leftover is not None
    # Correction DRs only for interior windows t=1..7: the two edge windows
    # (t=0, t=8) carry ~12% of the tap variance and dropping their hi*lo
    # corrections keeps absmax rel err ~1.9e-2 < 2e-2 while saving 2 DRs/half.
    c1c2 = [(t, ahat[t], col[t]) for t in range(1, 8)]
    return pairs, leftover, c1c2


def _build_program():
    if "nc" in _prog_cache:
        return _prog_cache["nc"]
    import concourse.bacc as bacc
    import concourse.mybir as mybir
    import concourse.tile as tile

    f8 = mybir.dt.float8e4
    f32 = mybir.dt.float32
    bf16 = mybir.dt.bfloat16
    DR = mybir.MatmulPerfMode.DoubleRow
    nc = bacc.Bacc("TRN2", target_bir_lowering=False, debug=False, num_devices=N_CORES)

    xin = nc.dram_tensor("xin", [128, 2, 8, 3, XCOL], f8, kind="ExternalInput")
    # 10 windows: win 9 duplicates win 8 so the odd-c leftover DR can pair
    # (Xh*Wh[8], Xl*Wh[8]) with a positive 16-aligned rhs slot stride.
    wshd = nc.dram_tensor("wshd", [128, 10, 2, 512], f8, kind="ExternalInput")
    out = nc.dram_tensor("out", [2, 8, 128, 512], bf16, kind="ExternalOutput")

    with tile.TileContext(nc) as tc:
        with (
            tc.tile_pool(name="const", bufs=1) as cpool,
            tc.tile_pool(name="stage", bufs=4) as spool,
            tc.tile_pool(name="psum", bufs=5, space="PSUM") as ppool,
            tc.tile_pool(name="psumn", bufs=2, space="PSUM") as npool,
            tc.tile_pool(name="psumw", bufs=1, space="PSUM") as wpool,
        ):
            x_sb = cpool.tile([128, 2, 8, 3, XCOL], f8, tag="x")
            wsh_sb = cpool.tile([128, 10, 2, 512], f8, tag="wsh")
            scr = cpool.tile([128, 128], f32, tag="scr")
            wps = wpool.tile([128, 512], f32, tag="wps", name="warm_ps")
            nc.vector.memset(scr[:], 0.0)
            for _w in range(N_WARM):
                nc.tensor.matmul(
                    out=wps[:, 0:128],
                    lhsT=scr[:],
                    rhs=scr[:],
                    start=(_w == 0),
                    stop=(_w == N_WARM - 1),
                )

            # --- input DMAs, ordered for delivery pacing.  SP issues them
            # (fastest sequencer + HWDGE constants); chunk-range x slices
            # keep >=512B contiguous runs (no descriptor penalty).  Wave 0
            # of phase 1 uses seg1 pairs (a_hat >= 4), so high chunks ship
            # first. ---
            nc.sync.dma_start(out=x_sb[:, 0, 4:8], in_=xin[:, 0, 4:8])
            nc.sync.dma_start(out=wsh_sb[:, 0:5, 1, :], in_=wshd[:, 0:5, 1, :])
            nc.sync.dma_start(out=x_sb[:, 0, 0:4], in_=xin[:, 0, 0:4])
            nc.sync.dma_start(out=wsh_sb[:, 5:10, 1, :], in_=wshd[:, 5:10, 1, :])
            nc.sync.dma_start(out=wsh_sb[:, 0:5, 0, :], in_=wshd[:, 0:5, 0, :])
            nc.sync.dma_start(out=wsh_sb[:, 5:10, 0, :], in_=wshd[:, 5:10, 0, :])
            nc.sync.dma_start(out=x_sb[:, 1], in_=xin[:, 1])

            def dr_main_pair(j, ps, t, ahat, col0, start):
                nc.tensor.matmul(
                    out=ps[:],
                    lhsT=x_sb[:, j, ahat : ahat + 2, 1, col0 : col0 + 128],
                    rhs=wsh_sb[:, t : t + 2, 1, :],
                    start=start,
                    stop=False,
                    perf_mode=DR,
                )

            def dr_c1c2(j, ps, t, ahat, col0):
                # slots (Xh*Wl, Xl*Wh)
                nc.tensor.matmul(
                    out=ps[:],
                    lhsT=x_sb[:, j, ahat, 1:3, col0 : col0 + 128],
                    rhs=wsh_sb[:, t, 0:2, :],
                    start=False,
                    stop=False,
                    perf_mode=DR,
                )

            def dr_leftover(j, ps, t, ahat, col0):
                if t == 8:
                    # odd c: slots (Xh*Wh[8], Xl*Wh[8]) — restores the x-side
                    # correction for the dropped edge window via the dup win 9
                    nc.tensor.matmul(
                        out=ps[:],
                        lhsT=x_sb[:, j, ahat, 1:3, col0 : col0 + 128],
                        rhs=wsh_sb[:, 8:10, 1, :],
                        start=False,
                        stop=True,
                        perf_mode=DR,
                    )
                else:
                    # even c: slots (Xl*Wl, Xh*Wh)
                    nc.tensor.matmul(
                        out=ps[:],
                        lhsT=x_sb[:, j, ahat, 0:2, col0 : col0 + 128],
                        rhs=wsh_sb[:, t, 0:2, :],
                        start=False,
                        stop=True,
                        perf_mode=DR,
                    )

            def drain(j, c, ps, eng):
                stage = spool.tile([128, 512], bf16, tag="stage", name=f"st{j}_{c}")
                if eng == 0:
                    nc.vector.tensor_copy(out=stage[:], in_=ps[:])
                else:
                    nc.scalar.copy(out=stage[:], in_=ps[:])
                nc.sync.dma_start(out=out[j, c], in_=stage[:])

            def emit_half_narrow(j, c, nlo, nhi, ps, eng):
                # full half restricted to psum/rhs columns [nlo, nhi) —
                # used to split the final half so its drain chain overlaps
                # the second sub-half's compute (shorter tail).
                pairs, leftover, c1c2 = plans[c]
                for w_idx, (t, ahat, col0) in enumerate(pairs):
                    nc.tensor.matmul(
                        out=ps[:],
                        lhsT=x_sb[:, j, ahat : ahat + 2, 1, col0 : col0 + 128],
                        rhs=wsh_sb[:, t : t + 2, 1, nlo:nhi],
                        start=(w_idx == 0),
                        stop=False,
                        perf_mode=DR,
                    )
                for t, ahat, col0 in c1c2:
                    nc.tensor.matmul(
                        out=ps[:],
                        lhsT=x_sb[:, j, ahat, 1:3, col0 : col0 + 128],
                        rhs=wsh_sb[:, t, 0:2, nlo:nhi],
                        start=False,
                        stop=False,
                        perf_mode=DR,
                    )
                t, ahat, col0 = leftover
                if t == 8:
                    nc.tensor.matmul(
                        out=ps[:],
                        lhsT=x_sb[:, j, ahat, 1:3, col0 : col0 + 128],
                        rhs=wsh_sb[:, 8:10, 1, nlo:nhi],
                        start=False,
                        stop=True,
                        perf_mode=DR,
                    )
                else:
                    nc.tensor.matmul(
                        out=ps[:],
                        lhsT=x_sb[:, j, ahat, 0:2, col0 : col0 + 128],
                        rhs=wsh_sb[:, t, 0:2, nlo:nhi],
                        start=False,
                        stop=True,
                        perf_mode=DR,
                    )
                stage = spool.tile(
                    [128, nhi - nlo], bf16, tag="stage2", name=f"stn{j}_{c}_{nlo}"
                )
                if eng == 0:
                    nc.vector.tensor_copy(out=stage[:], in_=ps[:])
                else:
                    nc.scalar.copy(out=stage[:], in_=ps[:])
                nc.sync.dma_start(out=out[j, c, :, nlo:nhi], in_=stage[:])

            plans = {c: _half_plan(c) for c in range(8)}
            halves = [(j, c) for j in range(2) for c in range(8)]
            # phase-1 wave order: c0's wave-0 pair is seg2 (low chunks, land
            # second), so rotate it last within each wave.
            grp_order = [halves[1], halves[2], halves[3], halves[0]][: GRP]

            ps_of = {}
            for (j, c) in grp_order:
                ps_of[(j, c)] = ppool.tile([128, 512], f32, tag="ps", name=f"ps{j}_{c}")
            for w_idx in range(4):
                for (j, c) in grp_order:
                    t, ahat, col0 = plans[c][0][w_idx]
                    dr_main_pair(j, ps_of[(j, c)], t, ahat, col0, start=(w_idx == 0))
            for t_idx in range(7):
                for (j, c) in grp_order:
                    t, ahat, col0 = plans[c][2][t_idx]
                    dr_c1c2(j, ps_of[(j, c)], t, ahat, col0)
            for n, (j, c) in enumerate(grp_order):
                t, ahat, col0 = plans[c][1]
                dr_leftover(j, ps_of[(j, c)], t, ahat, col0)
                drain(j, c, ps_of[(j, c)], n % 2)

            for n, (j, c) in enumerate(halves[GRP:]):
                pairs, leftover, c1c2 = plans[c]
                if (j, c) == halves[-1]:
                    psA = npool.tile([128, 256], f32, tag="psn", name="psA")
                    psB = npool.tile([128, 256], f32, tag="psn", name="psB")
                    emit_half_narrow(j, c, 0, 256, psA, 0)
                    emit_half_narrow(j, c, 256, 512, psB, 1)
                    continue
                ps = ppool.tile([128, 512], f32, tag="ps", name=f"ps{j}_{c}")
                for w_idx, (t, ahat, col0) in enumerate(pairs):
                    dr_main_pair(j, ps, t, ahat, col0, start=(w_idx == 0))
                for t, ahat, col0 in c1c2:
                    dr_c1c2(j, ps, t, ahat, col0)
                t, ahat, col0 = leftover
                dr_leftover(j, ps, t, ahat, col0)
                drain(j, c, ps, n % 2)

    nc.compile()
    _prog_cache["nc"] = nc
    return nc


def _quant_split(a, scale):
    """Return (hi, lo) fp8(e4m3) split of a*scale, as float8_e4m3 arrays."""
    import ml_dtypes

    f8 = ml_dtypes.float8_e4m3
    hi = (a * scale).astype(f8)
    lo = (a * scale - hi.astype(np.float32)).astype(f8)
    return hi, lo


def _host_pack(x, weight, sourceIdx):
    """Build per-core device inputs from full inputs."""
    import ml_dtypes

    f8 = ml_dtypes.float8_e4m3
    xh, xl = _quant_split(np.asarray(x, np.float32), XS)

    # scatter into blocked layout xs6[b, u_lo, i, a_hat, v3=(l,h,l), col=m+1]
    xs6 = np.zeros((B, 8, I, 8, 3, NBLK + 1), f8)
    src = np.asarray(sourceIdx, np.int64)
    for b in range(B):
        t = src[b]
        m = (t >> 6).astype(np.int64)
        u = (t & 63).astype(np.int64)
        ahat = 7 - (u >> 3)
        ulo = u & 7
        for i in range(I):
            xs6[b, ulo, i, ahat, 0, m + 1] = xl[b, i]
            xs6[b, ulo, i, ahat, 1, m + 1] = xh[b, i]
            xs6[b, ulo, i, ahat, 2, m + 1] = xl[b, i]

    x_cores = []
    for core in range(N_CORES):
        b, h = divmod(core, 2)
        tmp = xs6[b].reshape(128, 8, 3, NBLK + 1)
        arr = np.zeros((128, 2, 8, 3, XCOL), f8)
        for j in range(2):
            g = 2 * h + j
            arr[:, j, :, :, :129] = tmp[:, :, :, 128 * g : 128 * g + 129]
        x_cores.append(np.ascontiguousarray(arr))

    # shifted weights: wsh[p=(u',i), win, v2=(l,h), (zz',o)]
    wgt = np.asarray(weight, np.float32)  # (O, I, W)
    wh, wl = _quant_split(wgt, WS)
    zz = np.arange(72)
    up = np.arange(8)
    idx = zz[None, :] - up[:, None]              # (8 u', 72 zz')
    valid = (idx >= 0) & (idx < W)
    planes = []
    for wv in (wl, wh):
        g = wv.astype(np.float32)[:, :, np.clip(idx, 0, W - 1)] * valid[None, None]
        planes.append(g.transpose(2, 1, 3, 0).reshape(128, 9, 512))
    wsh_host = np.empty((128, 10, 2, 512), f8)
    wsh_host[:, :9, 0, :] = planes[0].astype(f8)
    wsh_host[:, :9, 1, :] = planes[1].astype(f8)
    wsh_host[:, 9] = wsh_host[:, 8]  # dup win 8 for the odd-c leftover DR
    return x_cores, np.ascontiguousarray(wsh_host)


def kernel(x, weight, bias, sourceIdx, nRealLen, _trace=False, _trace_out=None):
    import jax

    from concourse import bass_utils

    if len(jax.devices()) < N_CORES:
        jax.config.update("jax_platforms", "axon")
        try:
            import jax.extend.backend

            jax.extend.backend.clear_backends()
        except Exception:
            pass
        assert len(jax.devices()) >= N_CORES, (
            f"need {N_CORES} neuron cores, have {jax.devices()}"
        )

    nRealLen = int(nRealLen)
    assert nRealLen == T, f"kernel hardcoded for nRealLen={T}, got {nRealLen}"
    x_cores, wsh_host = _host_pack(x, weight, sourceIdx)
    nc = _build_program()
    in_maps = [{"xin": x_cores[c], "wshd": wsh_host} for c in range(N_CORES)]
    res = bass_utils.run_bass_kernel_spmd(
        nc,
        in_maps,
        core_ids=list(range(N_CORES)),
        trace=_trace,
        trace_cores=list(range(N_CORES)) if _trace else None,
    )
    if _trace_out is not None:
        _trace_out.append(res)
    bias_f = np.asarray(bias, np.float32)
    out_full = np.empty((B, O, T), np.float32)
    for core in range(N_CORES):
        b, h = divmod(core, 2)
        r = np.asarray(res.results[core]["out"], dtype=np.float32)  # (2,8,128,512)
        r6 = r.reshape(2, 8, 128, 8, 64)  # [j, c, m, q', o]
        for j in range(2):
            g = 2 * h + j
            # t' = m*64 + c*8 + q'
            seg = r6[j].transpose(3, 1, 0, 2).reshape(64, 8192)
            out_full[b, :, g * 8192 : (g + 1) * 8192] = seg
    out_full *= OUT_SCALE
    out_full += bias_f[None, :, None]
    return out_full

